# revision 2
# baseline (speedup 1.0000x reference)
"""Two-layer GCN (MultiOrderGraphLayer) Bass kernel for 8 Trainium2 cores.

Math: out = 0.5*(relu(A_hat@x@W1+b1) + relu(A_hat@x@W2+b2)) with
A_hat = D^-1/2 (A+I) D^-1/2.  Since both layers share A_hat, we compute
g = A_hat @ x once and apply the two small 128x128 matmuls afterwards.

Device algorithm (per core, feature-major layout g_T = [128 feat, nodes]):
  - nodes sharded 8 ways by row; edges partitioned by destination core.
  - self-loops are appended as ordinary edges with weight dinv[d]^2; every
    edge e carries norm_e = dinv[src]*dinv[dst] = sqrt(1/(deg[s]*deg[d])).
  - edges grouped per 128-node output window, padded to 128-edge blocks.
  - per block: dma_gather 128 rows of x (256B bf16 each); build the scaled
    one-hot S[e, n] = norm_e * (dstloc_e == n) with two DVE tensor_tensor
    ops (bf16 in/out for 2x DVE rate); accumulate t_T += xg^T @ S in PSUM.
  - dma_gather indices are int16, so sources are split into lo (<32768)
    and hi (>=32768) streams; each window is accumulated in two phases.
  - finish: out[n, fo] = relu(g_T^T @ (0.5*W) + 0.5*b) summed over layers,
    written node-major straight from PSUM-shaped matmuls (no transposes).

The whole block pipeline runs in bf16 (gathered x rows, one-hot, both
matmul operands); PSUM accumulation stays f32 and the final output is
written f32.  norm/dinv and the 0.5*W / 0.5*b folds are host-computed.
"""

import math
import numpy as np

N_NODES = 50000
D = 128
N_CORES = 8
SPLIT = 32768  # int16 gather index limit
WIN = 128      # output-window size in nodes (one-hot width / psum free dim)
CHUNK = 4096   # indices per dma_gather instruction (multiple of 128)
N_QUEUES = 4   # SWDGE queues; rotating queue_num 4x's gather bandwidth
GF = 8         # one-hot blocks fused per DVE tensor_tensor op
WARM = (1024, 1024, 2048, 2048)  # warmup chunk sizes (sum must stay in sync
                                 # with the split idx preload below)


# ---------------------------------------------------------------- host prep

def host_prep(edge_index, n_nodes, n_cores, split=SPLIT, chunk=CHUNK):
    """Index preprocessing: edge partitioning by destination, window
    grouping, lo/hi source split, padding, per-edge norm = dinv_s*dinv_d.

    Returns (meta, per_core_inputs) where per_core_inputs[c] is a dict of
    numpy arrays for core c's DRAM parameters (excluding x/W/b/iota).
    """
    src = np.asarray(edge_index[0], dtype=np.int64)
    dst = np.asarray(edge_index[1], dtype=np.int64)
    deg = np.bincount(dst, minlength=n_nodes).astype(np.int64) + 1

    loop = np.arange(n_nodes, dtype=np.int64)
    s_all = np.concatenate([src, loop])
    d_all = np.concatenate([dst, loop])
    norm_all = 1.0 / np.sqrt((deg[s_all] * deg[d_all]).astype(np.float64))
    norm_all = norm_all.astype(np.float32)

    npc = n_nodes // n_cores
    assert npc * n_cores == n_nodes
    nwin = math.ceil(npc / WIN)
    n_halves = 2 if n_nodes > split else 1

    per_core_sorted = []
    counts = np.zeros((n_cores, n_halves, nwin), np.int64)
    for c in range(n_cores):
        n0 = c * npc
        m = (d_all >= n0) & (d_all < n0 + npc)
        s, d, nr = s_all[m], d_all[m], norm_all[m]
        w = (d - n0) // WIN
        half = (s >= split).astype(np.int64) if n_halves == 2 else np.zeros_like(s)
        key = half * nwin + w
        order = np.argsort(key, kind="stable")
        s, d, nr, key = s[order], d[order], nr[order], key[order]
        cnt = np.bincount(key, minlength=n_halves * nwin)
        counts[c] = cnt.reshape(n_halves, nwin)
        per_core_sorted.append((s, d, nr, cnt))

    # shared block structure: blocks per (half, window), equal across cores
    nblk = np.maximum(1, -(-counts.max(axis=0) // 128))  # [n_halves, nwin]
    half_tot = nblk.sum(axis=1) * 128                    # edge slots per half
    nblk_tot = int(nblk.sum())

    # chunk split per half (shared across cores); small warmup chunks so
    # the first PSUM windows start within ~15us instead of waiting for a
    # full gather to drain, and small cooldown chunks to shorten the tail
    chunk_sizes = []
    for h in range(n_halves):
        rem, sizes = int(half_tot[h]), []
        for w in WARM:
            L = min(w, rem)
            if L > 0:
                sizes.append(L)
                rem -= L
        while rem > 0:
            L = min(chunk, rem)
            sizes.append(L)
            rem -= L
        if sizes and sizes[-1] == chunk:
            sizes[-1:] = [chunk // 2, chunk // 4, chunk // 4]
        chunk_sizes.append(sizes)

    per_core_inputs = []
    for c in range(n_cores):
        s, d, nr, cnt = per_core_sorted[c]
        offs = np.concatenate([[0], np.cumsum(cnt)])
        idx_h = [[] for _ in range(n_halves)]
        dl_parts, dv_parts = [], []
        for h in range(n_halves):
            for wi in range(nwin):
                k = h * nwin + wi
                a, b = int(offs[k]), int(offs[k + 1])
                L = int(nblk[h, wi]) * 128
                pad = L - (b - a)
                gs = np.concatenate([s[a:b] - h * split,
                                     np.zeros(pad, np.int64)])
                gd = np.concatenate([(d[a:b] - c * npc - wi * WIN).astype(np.float32),
                                     np.full(pad, -1.0, np.float32)])
                gv = np.concatenate([nr[a:b], np.ones(pad, np.float32)])
                idx_h[h].append(gs.astype(np.int16))
                dl_parts.append(gd)
                dv_parts.append(gv)

        # one-hot metadata, block-major -> [128 lanes, nblk_tot]
        dl_stream = np.concatenate(dl_parts).reshape(-1, 128)
        dv_stream = np.concatenate(dv_parts).reshape(-1, 128)
        core_in = {
            "dstloc": np.ascontiguousarray(dl_stream.T),
            "dinv": np.ascontiguousarray(dv_stream.T),
        }
        # gather indices: wrapped [16, L/16] per chunk, replicated to 128 rows
        for h in range(n_halves):
            stream = np.concatenate(idx_h[h])
            cols, off = [], 0
            for L in chunk_sizes[h]:
                a = stream[off:off + L].reshape(-1, 16).T  # [16, L/16]
                cols.append(a)
                off += L
            wrapped = np.concatenate(cols, axis=1)         # [16, half_tot/16]
            core_in["idx_h%d" % h] = np.ascontiguousarray(
                np.tile(wrapped, (8, 1)))
        per_core_inputs.append(core_in)

    meta = dict(n_nodes=n_nodes, n_cores=n_cores, npc=npc, nwin=nwin,
                n_halves=n_halves, split=split, nblk=nblk,
                half_tot=half_tot, nblk_tot=nblk_tot, chunk=chunk,
                chunk_sizes=chunk_sizes)
    return meta, per_core_inputs


# ------------------------------------------------------------- bass program

def build_program(meta):
    import concourse.bacc as bacc
    import concourse.mybir as mybir
    import concourse.tile as tile
    from concourse import library_config

    f32 = mybir.dt.float32
    bf16 = mybir.dt.bfloat16
    i16 = mybir.dt.int16
    AF = mybir.ActivationFunctionType
    OP = mybir.AluOpType

    n_nodes = meta["n_nodes"]
    npc, nwin = meta["npc"], meta["nwin"]
    n_halves, split = meta["n_halves"], meta["split"]
    nblk, nblk_tot = meta["nblk"], meta["nblk_tot"]
    chunk = meta["chunk"]
    chunk_sizes = meta["chunk_sizes"]
    warm_tot = sum(WARM)

    nc = bacc.Bacc("TRN2", num_swdge_queues=N_QUEUES)

    x_d = nc.declare_dram_parameter("x", [n_nodes, D], bf16, isOutput=False)
    dl_d = nc.declare_dram_parameter("dstloc", [128, nblk_tot], bf16, isOutput=False)
    dv_d = nc.declare_dram_parameter("dinv", [128, nblk_tot], bf16, isOutput=False)
    idx_d = [nc.declare_dram_parameter("idx_h%d" % h,
                                       [128, int(meta["half_tot"][h]) // 16],
                                       i16, isOutput=False)
             for h in range(n_halves)]
    w1_d = nc.declare_dram_parameter("W1", [D, D], bf16, isOutput=False)
    w2_d = nc.declare_dram_parameter("W2", [D, D], bf16, isOutput=False)
    b1_d = nc.declare_dram_parameter("b1", [1, D], bf16, isOutput=False)
    b2_d = nc.declare_dram_parameter("b2", [1, D], bf16, isOutput=False)
    iota_d = nc.declare_dram_parameter("iota", [128, GF * 128], bf16,
                                       isOutput=False)
    out_d = nc.declare_dram_parameter("out", [npc, D], f32, isOutput=True)

    WG = 4  # windows per phase-2 batch (one 512-wide psum bank)

    with tile.TileContext(nc) as tc:
        with (
            tc.tile_pool(name="const", bufs=1) as constp,
            tc.tile_pool(name="xg", bufs=6) as xgp,
            tc.tile_pool(name="eq", bufs=3) as eqp,
            tc.tile_pool(name="oh", bufs=4) as ohp,
            tc.tile_pool(name="ps1", bufs=3, space="PSUM") as ps1,
            tc.tile_pool(name="ps2", bufs=2, space="PSUM") as ps2,
            tc.tile_pool(name="fin", bufs=3) as finp,
        ):
            # Q7 library holding DMAGatherAnt; must precede all gathers
            nc.gpsimd.load_library(library_config.mlp)

            # --- constants / metadata (weights and biases arrive already
            # scaled by 0.5; dinv is host-computed)
            iota8 = constp.tile([128, GF, 128], bf16)
            nc.sync.dma_start(
                iota8[:], iota_d[:].rearrange("p (c n) -> p c n", n=128))
            wts = {}
            for nm, src_d in (("w1", w1_d), ("w2", w2_d)):
                t = constp.tile([128, 128], bf16, tag=nm)
                nc.sync.dma_start(t[:], src_d[:])
                wts[nm] = t
            bias = {}
            for nm, src_d in (("b1", b1_d), ("b2", b2_d)):
                t = constp.tile([1, 128], bf16, tag=nm)
                nc.sync.dma_start(t[:], src_d[:])
                bias[nm] = t
            ones = constp.tile([1, 128], bf16)
            nc.vector.memset(ones[:], 1.0)

            dl = constp.tile([128, nblk_tot], bf16)
            nc.sync.dma_start(dl[:], dl_d[:])
            dinv = constp.tile([128, nblk_tot], bf16)
            nc.sync.dma_start(dinv[:], dv_d[:])

            g_all = constp.tile([128, npc], bf16)

            # idx streams: split the preload so the warmup chunks' indices
            # land quickly and the first gather starts ~15us earlier
            idx_all = []
            for h in range(n_halves):
                cols = int(meta["half_tot"][h]) // 16
                wcols = min(warm_tot // 16, cols)
                t = constp.tile([128, cols], i16, tag="idx%d" % h)
                nc.sync.dma_start(t[:, :wcols], idx_d[h][:, :wcols])
                if wcols < cols:
                    nc.sync.dma_start(t[:, wcols:], idx_d[h][:, wcols:])
                idx_all.append(t)

            # one-hot groups: GF blocks fused per DVE op; one active
            # group cached per half (streams are consumed interleaved)
            oh_cache = {}

            def get_oh(bg, h):
                g = bg // GF
                if oh_cache.get(h, (None, None))[0] != g:
                    g0 = g * GF
                    gl = min(GF, nblk_tot - g0)
                    eq = eqp.tile([128, GF, 128], bf16, tag="eq")
                    nc.vector.tensor_tensor(
                        out=eq[:, :gl, :], in0=iota8[:, :gl, :],
                        in1=dl[:, g0:g0 + gl, None].to_broadcast([128, gl, 128]),
                        op=OP.is_equal)
                    oh = ohp.tile([128, GF, 128], bf16, tag="oh")
                    nc.vector.tensor_tensor(
                        out=oh[:, :gl, :], in0=eq[:, :gl, :],
                        in1=dinv[:, g0:g0 + gl, None].to_broadcast([128, gl, 128]),
                        op=OP.mult)
                    oh_cache[h] = (g, oh)
                return oh_cache[h][1]

            # per-half stream state: lazy chunk issuing in window order
            class Stream:
                pass

            streams = []
            blk_base = 0
            for h in range(n_halves):
                s = Stream()
                s.h = h
                s.base = x_d[0:split, :] if h == 0 else x_d[split:n_nodes, :]
                s.wstart = np.concatenate([[0], np.cumsum(nblk[h])])
                s.blk_base = blk_base          # global block id of stream pos 0
                s.chunk_bounds = []
                off = 0
                for L in chunk_sizes[h]:
                    s.chunk_bounds.append((off, L))
                    off += L
                s.blk2chunk = np.repeat(
                    np.arange(len(chunk_sizes[h])),
                    [L // 128 for L in chunk_sizes[h]])
                s.tiles = {}
                blk_base += int(nblk[h].sum())
                streams.append(s)

            ci_global = 0

            def ensure_chunk(s, ci):
                nonlocal ci_global
                if ci in s.tiles:
                    return s.tiles[ci]
                off, L = s.chunk_bounds[ci]
                xg = xgp.tile([128, chunk // 128, 128], bf16, tag="xg")
                nc.gpsimd.dma_gather(
                    out_ap=xg[:, : L // 128, :],
                    in_ap=s.base,
                    idxs_ap=idx_all[s.h][:, off // 16:(off + L) // 16],
                    num_idxs=L,
                    num_idxs_reg=L,
                    elem_size=D,
                    single_packet=False,
                    queue_num=ci_global % N_QUEUES,
                )
                ci_global += 1
                s.tiles.clear()
                s.tiles[ci] = xg
                return xg

            # --- fused pass: per window accumulate lo+hi edge blocks in
            # one PSUM group, flush to g_all; every WG windows run the
            # output stage (overlaps with later windows' aggregation).
            def emit_phase2(wlo, whi):
                nwg = whi - wlo + 1
                wls = [min(WIN, npc - w * WIN) for w in range(wlo, whi + 1)]
                rows = min(wls)  # < 128 only for a trailing ragged window
                pps = {}
                for nm_w, nm_b in (("w1", "b1"), ("w2", "b2")):
                    pp = ps2.tile([128, WG * 128], f32, tag="pp")
                    for j, w in enumerate(range(wlo, whi + 1)):
                        wl = wls[j]
                        sl = pp[:wl, j * 128:(j + 1) * 128]
                        nc.tensor.matmul(sl, g_all[:, w * WIN:w * WIN + wl],
                                         wts[nm_w][:], start=True, stop=False)
                        nc.tensor.matmul(sl, ones[:, :wl], bias[nm_b][:],
                                         start=False, stop=True)
                    o = finp.tile([128, WG, 128], f32, tag="o" + nm_w)
                    of = o[:].rearrange("p c n -> p (c n)")
                    if rows == 128:
                        nc.scalar.activation(of[:, :nwg * 128],
                                             pp[:, :nwg * 128], AF.Relu)
                    else:
                        for j in range(nwg):
                            nc.scalar.activation(
                                of[:wls[j], j * 128:(j + 1) * 128],
                                pp[:wls[j], j * 128:(j + 1) * 128], AF.Relu)
                    pps[nm_w] = o
                ot = finp.tile([128, WG, 128], f32, tag="ot")
                otf = ot[:].rearrange("p c n -> p (c n)")
                o1f = pps["w1"][:].rearrange("p c n -> p (c n)")
                o2f = pps["w2"][:].rearrange("p c n -> p (c n)")
                if rows == 128:
                    nc.vector.tensor_tensor(otf[:, :nwg * 128],
                                            o1f[:, :nwg * 128],
                                            o2f[:, :nwg * 128], op=OP.add)
                else:
                    for j in range(nwg):
                        cs = slice(j * 128, j * 128 + 128)
                        nc.vector.tensor_tensor(otf[:wls[j], cs],
                                                o1f[:wls[j], cs],
                                                o2f[:wls[j], cs], op=OP.add)
                for j, w in enumerate(range(wlo, whi + 1)):
                    nc.sync.dma_start(out_d[w * WIN:w * WIN + wls[j], :],
                                      ot[:wls[j], j, :])

            for w in range(nwin):
                wlen = min(WIN, npc - w * WIN)
                pw = ps1.tile([128, 128], f32, tag="pw")
                # total blocks this window across halves
                runs = []
                for s in streams:
                    b0, b1 = int(s.wstart[w]), int(s.wstart[w + 1])
                    runs.append((s, b0, b1))
                n_tot = sum(b1 - b0 for _, b0, b1 in runs)
                k = 0
                for s, b0, b1 in runs:
                    for b in range(b0, b1):
                        ci = int(s.blk2chunk[b])
                        xg = ensure_chunk(s, ci)
                        bl = (b * 128 - s.chunk_bounds[ci][0]) // 128
                        bg = s.blk_base + b
                        oh = get_oh(bg, s.h)
                        nc.tensor.matmul(
                            pw[:, :wlen],
                            xg[:, bl, :],
                            oh[:, bg % GF, :wlen],
                            start=(k == 0),
                            stop=(k == n_tot - 1),
                        )
                        k += 1
                nc.scalar.activation(g_all[:, w * WIN:w * WIN + wlen],
                                     pw[:, :wlen], AF.Copy)
                if w % WG == WG - 1 or w == nwin - 1:
                    emit_phase2(w - (w % WG), w)

    nc.compile()
    return nc


def make_core_inputs(meta, per_core_inputs, x, W1, b1, W2, b2):
    """Full in_maps for run_bass_kernel_spmd (adds shared tensors)."""
    import ml_dtypes
    bf = ml_dtypes.bfloat16
    x = np.ascontiguousarray(np.asarray(x, np.float32).astype(bf))
    shared = {
        "x": x,
        "W1": np.ascontiguousarray((0.5 * np.asarray(W1, np.float32)).astype(bf)),
        "W2": np.ascontiguousarray((0.5 * np.asarray(W2, np.float32)).astype(bf)),
        "b1": (0.5 * np.asarray(b1, np.float32)).astype(bf).reshape(1, D),
        "b2": (0.5 * np.asarray(b2, np.float32)).astype(bf).reshape(1, D),
        "iota": np.ascontiguousarray(np.broadcast_to(
            np.tile(np.arange(128, dtype=np.float32), GF),
            (128, GF * 128)).astype(bf)),
    }
    maps = []
    for ci in per_core_inputs:
        m = dict(shared)
        m["dstloc"] = np.ascontiguousarray(ci["dstloc"].astype(bf))
        m["dinv"] = np.ascontiguousarray(ci["dinv"].astype(bf))
        for k, v in ci.items():
            if k.startswith("idx_"):
                m[k] = v
        maps.append(m)
    return maps


# ------------------------------------------------------------------- kernel

def kernel(x, edge_index, W1, b1, W2, b2, _trace=False):
    from concourse.bass_utils import run_bass_kernel_spmd

    x = np.asarray(x)
    n_nodes = x.shape[0]
    meta, pci = host_prep(edge_index, n_nodes, N_CORES)
    nc = build_program(meta)
    in_maps = make_core_inputs(meta, pci, x, W1, b1, W2, b2)
    res = run_bass_kernel_spmd(nc, in_maps, list(range(N_CORES)),
                               trace=_trace)
    out = np.concatenate([res.results[c]["out"] for c in range(N_CORES)],
                         axis=0)
    if _trace:
        return out, res
    return out


# revision 6
# speedup vs baseline: 2.4131x; 2.4131x over previous
"""Two-layer GCN (MultiOrderGraphLayer) Bass kernel for 8 Trainium2 cores.

Math: out = 0.5*(relu(A_hat@x@W1+b1) + relu(A_hat@x@W2+b2)) with
A_hat = D^-1/2 (A+I) D^-1/2.  Both layers share A_hat, so g = A_hat @ x is
computed once and the two small 128x128 matmuls are applied afterwards.

Normalization factorization: norm_e = dinv[src]*dinv[dst].  The host
pre-scales x rows by dinv (x' = dinv[i]*x[i], bf16), so gathered rows
already carry the src factor; the dst factor dinv[n] is applied in the
output stage as a per-partition activation scale (partition = node there).
Self-loops reduce to g_raw[:, n] += x'[n, :], added per window with one
identity matmul instead of gather slots.

Device algorithm (per core, feature-major g_T = [128 feat, nodes]):
  - nodes sharded 8 ways by row; edges partitioned by destination core and
    grouped per 128-node output window into variable-length cells packed
    back-to-back (cell length = max edge count across cores, so the SPMD
    program shape is shared; per-core shortfall is masked via dstloc=-1).
  - sources split into lo (<32768) / hi streams for int16 dma_gather.
  - the stream is consumed in fixed 128-slot blocks; for each (window,
    overlapping block) pair the host emits a dstloc column; slots outside
    the window (straddle or pad) carry -1 and the one-hot masks them out.
  - per block-use: build S[e, n] = (dstloc_e == n) in one DVE is_equal
    (bf16, GF columns fused); accumulate psum += xg^T @ S.
  - output: psum = g_T^T@(0.5W) + rdinv*(0.5b); out = relu(psum * dinv[n])
    (per-partition scale), layers averaged, written node-major.
"""

import math
import numpy as np

N_NODES = 50000
D = 128
N_CORES = 8
SPLIT = 32768  # int16 gather index limit
WIN = 128      # output-window size in nodes (one-hot width / psum free dim)
CHUNK = 4096   # slots per dma_gather instruction (multiple of 128)
N_QUEUES = 4   # SWDGE queues (ucode max); rotation parallelizes drain
GF = 8         # one-hot columns fused per DVE is_equal op
WARM = (1024, 1024, 2048, 2048)  # warmup chunks; keep in sync with idx split


# ---------------------------------------------------------------- host prep

def host_prep(edge_index, n_nodes, n_cores, split=SPLIT, chunk=CHUNK):
    """Edge partitioning by destination core, per-window cells (variable
    length, shared shape across cores), lo/hi source split, block/window
    overlap map, dstloc mask columns.

    All heavy math stays on device; host work is indexing plus the x
    prescale (done in make_core_inputs).
    """
    src = np.asarray(edge_index[0], dtype=np.int64)
    dst = np.asarray(edge_index[1], dtype=np.int64)
    deg = np.bincount(dst, minlength=n_nodes).astype(np.int64) + 1
    dinv = (1.0 / np.sqrt(deg.astype(np.float64))).astype(np.float32)

    npc = n_nodes // n_cores
    assert npc * n_cores == n_nodes
    nwin = math.ceil(npc / WIN)
    n_halves = 2 if n_nodes > split else 1

    per_core = []
    counts = np.zeros((n_cores, n_halves, nwin), np.int64)
    for c in range(n_cores):
        n0 = c * npc
        m = (dst >= n0) & (dst < n0 + npc)
        s, d = src[m], dst[m]
        w = (d - n0) // WIN
        half = (s >= split).astype(np.int64) if n_halves == 2 else np.zeros_like(s)
        key = half * nwin + w
        order = np.argsort(key, kind="stable")
        s, d, key = s[order], d[order], key[order]
        cnt = np.bincount(key, minlength=n_halves * nwin)
        counts[c] = cnt.reshape(n_halves, nwin)
        per_core.append((s, d, cnt))

    # shared cell lengths (max across cores) and packed offsets per half
    clen = counts.max(axis=0)                       # [n_halves, nwin]
    coff = np.zeros_like(clen)
    half_len = np.zeros(n_halves, np.int64)
    for h in range(n_halves):
        coff[h] = np.concatenate([[0], np.cumsum(clen[h])[:-1]])
        half_len[h] = -(-int(clen[h].sum()) // 128) * 128  # pad tail to 128

    # block/window overlap map (compile-time, shared across cores):
    # mm list per window = blocks intersecting [coff, coff+clen)
    mm_cols = []   # global column order: for w: lo block uses, hi block uses
    win_mms = [[] for _ in range(nwin)]
    for w in range(nwin):
        for h in range(n_halves):
            a, b = int(coff[h, w]), int(coff[h, w] + clen[h, w])
            if b == a:
                continue
            for blk in range(a // 128, -(-b // 128)):
                win_mms[w].append((h, blk, len(mm_cols)))
                mm_cols.append((h, blk, w))
    nmm = len(mm_cols)

    # chunk split per half: warmup small, bulk CHUNK, cooldown small
    chunk_sizes = []
    for h in range(n_halves):
        rem, sizes = int(half_len[h]), []
        for wsz in WARM:
            L = min(wsz, rem)
            if L > 0:
                sizes.append(L)
                rem -= L
        while rem > 0:
            L = min(chunk, rem)
            sizes.append(L)
            rem -= L
        if sizes and sizes[-1] == chunk:
            sizes[-1:] = [chunk // 2, chunk // 4, chunk // 4]
        chunk_sizes.append(sizes)

    per_core_inputs = []
    for c in range(n_cores):
        s, d, cnt = per_core[c]
        offs = np.concatenate([[0], np.cumsum(cnt)])
        # slot streams per half: sources (idx) and per-slot local dst
        idx_stream = [np.zeros(int(half_len[h]), np.int64) for h in range(n_halves)]
        dst_stream = [np.full(int(half_len[h]), -1, np.int64) for h in range(n_halves)]
        for h in range(n_halves):
            for w in range(nwin):
                k = h * nwin + w
                a, b = int(offs[k]), int(offs[k + 1])
                o = int(coff[h, w])
                idx_stream[h][o:o + (b - a)] = s[a:b] - h * split
                dst_stream[h][o:o + (b - a)] = d[a:b] - c * npc - w * WIN

        # dstloc mask columns: one [128] column per (window, block) use;
        # slots outside the window's cell get -1
        dl = np.full((128, nmm), -1.0, np.float32)
        for col, (h, blk, w) in enumerate(mm_cols):
            s0, s1 = blk * 128, blk * 128 + 128
            a, b = int(coff[h, w]), int(coff[h, w] + clen[h, w])
            lo, hi = max(s0, a), min(s1, b)
            if hi > lo:
                seg = dst_stream[h][lo:hi].astype(np.float32)
                # mask slots whose dst is outside this window (pad slots
                # carry -1 already; straddle slots belong to w by range)
                dl[lo - s0:hi - s0, col] = seg
        core_in = {"dstloc": np.ascontiguousarray(dl)}

        # own-slab metadata for self-loops + output normalization
        nd = np.arange(npc, dtype=np.int64) + c * npc
        dv = dinv[nd]                                   # [npc]
        pad = nwin * WIN - npc
        dvp = np.concatenate([dv, np.zeros(pad, np.float32)])
        core_in["dinv_own"] = np.ascontiguousarray(
            dvp.reshape(nwin, WIN).T)                   # [128, nwin]
        core_in["rdinv"] = (1.0 / dvp.reshape(1, -1)[:, :npc].clip(1e-30)
                            ).astype(np.float32)        # [1, npc]

        # gather indices: wrapped [16, L/16] per chunk, replicated 8x
        for h in range(n_halves):
            cols, off = [], 0
            for L in chunk_sizes[h]:
                a = idx_stream[h][off:off + L].reshape(-1, 16).T
                cols.append(a)
                off += L
            wrapped = np.concatenate(cols, axis=1).astype(np.int16)
            core_in["idx_h%d" % h] = np.ascontiguousarray(
                np.tile(wrapped, (8, 1)))
        per_core_inputs.append(core_in)

    meta = dict(n_nodes=n_nodes, n_cores=n_cores, npc=npc, nwin=nwin,
                n_halves=n_halves, split=split, half_len=half_len,
                nmm=nmm, win_mms=win_mms, chunk=chunk,
                chunk_sizes=chunk_sizes, dinv=dinv)
    return meta, per_core_inputs


# ------------------------------------------------------------- bass program

def build_program(meta):
    import concourse.bacc as bacc
    import concourse.mybir as mybir
    import concourse.tile as tile
    from concourse import library_config

    f32 = mybir.dt.float32
    bf16 = mybir.dt.bfloat16
    i16 = mybir.dt.int16
    AF = mybir.ActivationFunctionType
    OP = mybir.AluOpType

    n_nodes = meta["n_nodes"]
    npc, nwin = meta["npc"], meta["nwin"]
    n_halves, split = meta["n_halves"], meta["split"]
    half_len = meta["half_len"]
    nmm = meta["nmm"]
    win_mms = meta["win_mms"]
    chunk = meta["chunk"]
    chunk_sizes = meta["chunk_sizes"]
    warm_tot = sum(WARM)

    nc = bacc.Bacc("TRN2", num_swdge_queues=N_QUEUES)

    x_d = nc.declare_dram_parameter("x", [n_nodes, D], bf16, isOutput=False)
    xo_d = nc.declare_dram_parameter("x_own", [npc, D], bf16, isOutput=False)
    dl_d = nc.declare_dram_parameter("dstloc", [128, nmm], bf16, isOutput=False)
    dvo_d = nc.declare_dram_parameter("dinv_own", [128, nwin], f32,
                                      isOutput=False)
    rd_d = nc.declare_dram_parameter("rdinv", [1, npc], bf16, isOutput=False)
    idx_d = [nc.declare_dram_parameter("idx_h%d" % h,
                                       [128, int(half_len[h]) // 16],
                                       i16, isOutput=False)
             for h in range(n_halves)]
    w1_d = nc.declare_dram_parameter("W1", [D, D], bf16, isOutput=False)
    w2_d = nc.declare_dram_parameter("W2", [D, D], bf16, isOutput=False)
    b1_d = nc.declare_dram_parameter("b1", [1, D], bf16, isOutput=False)
    b2_d = nc.declare_dram_parameter("b2", [1, D], bf16, isOutput=False)
    iota_d = nc.declare_dram_parameter("iota", [128, GF * 128], bf16,
                                       isOutput=False)
    id_d = nc.declare_dram_parameter("ident", [128, 128], bf16, isOutput=False)
    out_d = nc.declare_dram_parameter("out", [npc, D], f32, isOutput=True)

    WG = 4  # windows per phase-2 batch (one 512-wide psum bank)

    with tile.TileContext(nc) as tc:
        with (
            tc.tile_pool(name="const", bufs=1) as constp,
            tc.tile_pool(name="xg", bufs=8) as xgp,
            tc.tile_pool(name="eq", bufs=4) as eqp,
            tc.tile_pool(name="ps1", bufs=3, space="PSUM") as ps1,
            tc.tile_pool(name="ps2", bufs=2, space="PSUM") as ps2,
            tc.tile_pool(name="fin", bufs=3) as finp,
        ):
            # Q7 library holding DMAGatherAnt; must precede all gathers
            nc.gpsimd.load_library(library_config.mlp)

            # --- constants / metadata (weights and biases arrive already
            # scaled by 0.5; x rows arrive scaled by dinv[src])
            iota8 = constp.tile([128, GF, 128], bf16)
            nc.sync.dma_start(
                iota8[:], iota_d[:].rearrange("p (c n) -> p c n", n=128))
            ident = constp.tile([128, 128], bf16)
            nc.sync.dma_start(ident[:], id_d[:])
            wts = {}
            for nm, src_d in (("w1", w1_d), ("w2", w2_d)):
                t = constp.tile([128, 128], bf16, tag=nm)
                nc.sync.dma_start(t[:], src_d[:])
                wts[nm] = t
            bias = {}
            for nm, src_d in (("b1", b1_d), ("b2", b2_d)):
                t = constp.tile([1, 128], bf16, tag=nm)
                nc.sync.dma_start(t[:], src_d[:])
                bias[nm] = t
            rdinv = constp.tile([1, npc], bf16)
            nc.sync.dma_start(rdinv[:], rd_d[:])
            dvo = constp.tile([128, nwin], f32)
            nc.sync.dma_start(dvo[:], dvo_d[:])

            dl = constp.tile([128, nmm], bf16)
            nc.sync.dma_start(dl[:], dl_d[:])

            # own slab, node-major per window: [128 node, nwin, 128 feat]
            xown = constp.tile([128, nwin, 128], bf16)
            nc.sync.dma_start(
                xown[:, :npc // 128, :],
                xo_d[: (npc // 128) * 128, :].rearrange(
                    "(w p) f -> p w f", p=128))
            if npc % 128:
                nc.sync.dma_start(
                    xown[: npc % 128, npc // 128, :],
                    xo_d[(npc // 128) * 128:, :])

            g_all = constp.tile([128, npc], bf16)

            # idx streams: split the preload so the warmup chunks' indices
            # land quickly and the first gather starts early
            idx_all = []
            for h in range(n_halves):
                cols = int(half_len[h]) // 16
                wcols = min(warm_tot // 16, cols)
                t = constp.tile([128, cols], i16, tag="idx%d" % h)
                nc.sync.dma_start(t[:, :wcols], idx_d[h][:, :wcols])
                if wcols < cols:
                    nc.sync.dma_start(t[:, wcols:], idx_d[h][:, wcols:])
                idx_all.append(t)

            # one-hot columns: GF fused per DVE is_equal; consumed strictly
            # in column order so a single active group suffices
            eq_cache = [None, None]  # [group id, tile]

            def get_eq(col):
                g = col // GF
                if eq_cache[0] != g:
                    g0 = g * GF
                    gl = min(GF, nmm - g0)
                    eq = eqp.tile([128, GF, 128], bf16, tag="eq")
                    nc.vector.tensor_tensor(
                        out=eq[:, :gl, :], in0=iota8[:, :gl, :],
                        in1=dl[:, g0:g0 + gl, None].to_broadcast([128, gl, 128]),
                        op=OP.is_equal)
                    eq_cache[0] = g
                    eq_cache[1] = eq
                return eq_cache[1]

            # per-half stream state: lazy chunk issuing in window order
            class Stream:
                pass

            streams = []
            for h in range(n_halves):
                s = Stream()
                s.h = h
                s.base = x_d[0:split, :] if h == 0 else x_d[split:n_nodes, :]
                s.chunk_bounds = []
                off = 0
                for L in chunk_sizes[h]:
                    s.chunk_bounds.append((off, L))
                    off += L
                s.blk2chunk = np.repeat(
                    np.arange(len(chunk_sizes[h])),
                    [L // 128 for L in chunk_sizes[h]])
                s.tiles = {}
                streams.append(s)

            ci_global = 0

            def ensure_chunk(s, ci):
                nonlocal ci_global
                if ci in s.tiles:
                    return s.tiles[ci]
                off, L = s.chunk_bounds[ci]
                xg = xgp.tile([128, chunk // 128, 128], bf16, tag="xg")
                nc.gpsimd.dma_gather(
                    out_ap=xg[:, : L // 128, :],
                    in_ap=s.base,
                    idxs_ap=idx_all[s.h][:, off // 16:(off + L) // 16],
                    num_idxs=L,
                    num_idxs_reg=L,
                    elem_size=D,
                    single_packet=False,
                    queue_num=ci_global % N_QUEUES,
                )
                ci_global += 1
                s.tiles.clear()
                s.tiles[ci] = xg
                return xg

            # --- output stage: psum = g^T@(W/2) + rdinv*(b/2), then
            # relu with per-partition dst scale dinv[n]; layers averaged
            def emit_phase2(wlo, whi):
                nwg = whi - wlo + 1
                wls = [min(WIN, npc - w * WIN) for w in range(wlo, whi + 1)]
                pps = {}
                for nm_w, nm_b in (("w1", "b1"), ("w2", "b2")):
                    pp = ps2.tile([128, WG * 128], f32, tag="pp")
                    for j, w in enumerate(range(wlo, whi + 1)):
                        wl = wls[j]
                        sl = pp[:wl, j * 128:(j + 1) * 128]
                        nc.tensor.matmul(sl, g_all[:, w * WIN:w * WIN + wl],
                                         wts[nm_w][:], start=True, stop=False)
                        nc.tensor.matmul(sl,
                                         rdinv[:, w * WIN:w * WIN + wl],
                                         bias[nm_b][:], start=False, stop=True)
                    o = finp.tile([128, WG, 128], f32, tag="o" + nm_w)
                    for j, w in enumerate(range(wlo, whi + 1)):
                        nc.scalar.activation(
                            o[:wls[j], j, :],
                            pp[:wls[j], j * 128:(j + 1) * 128], AF.Relu,
                            scale=dvo[:wls[j], w:w + 1])
                    pps[nm_w] = o
                ot = finp.tile([128, WG, 128], f32, tag="ot")
                rows = min(wls)
                otf = ot[:].rearrange("p c n -> p (c n)")
                o1f = pps["w1"][:].rearrange("p c n -> p (c n)")
                o2f = pps["w2"][:].rearrange("p c n -> p (c n)")
                if rows == 128:
                    nc.vector.tensor_tensor(otf[:, :nwg * 128],
                                            o1f[:, :nwg * 128],
                                            o2f[:, :nwg * 128], op=OP.add)
                else:
                    for j in range(nwg):
                        cs = slice(j * 128, j * 128 + 128)
                        nc.vector.tensor_tensor(otf[:wls[j], cs],
                                                o1f[:wls[j], cs],
                                                o2f[:wls[j], cs], op=OP.add)
                for j, w in enumerate(range(wlo, whi + 1)):
                    nc.sync.dma_start(out_d[w * WIN:w * WIN + wls[j], :],
                                      ot[:wls[j], j, :])

            for w in range(nwin):
                wlen = min(WIN, npc - w * WIN)
                pw = ps1.tile([128, 128], f32, tag="pw")
                mms = win_mms[w]
                n_tot = len(mms) + 1
                # self-loop first: x'_own rows -> columns via identity
                nc.tensor.matmul(pw[:, :wlen], xown[:wlen, w, :],
                                 ident[:wlen, :wlen],
                                 start=True, stop=(n_tot == 1))
                for k, (h, blk, col) in enumerate(mms):
                    s = streams[h]
                    ci = int(s.blk2chunk[blk])
                    xg = ensure_chunk(s, ci)
                    bl = blk - s.chunk_bounds[ci][0] // 128
                    eq = get_eq(col)
                    nc.tensor.matmul(
                        pw[:, :wlen],
                        xg[:, bl, :],
                        eq[:, col % GF, :wlen],
                        start=False,
                        stop=(k == n_tot - 2),
                    )
                nc.scalar.activation(g_all[:, w * WIN:w * WIN + wlen],
                                     pw[:, :wlen], AF.Copy)
                if w % WG == WG - 1 or w == nwin - 1:
                    emit_phase2(w - (w % WG), w)

    nc.compile()
    return nc


def make_core_inputs(meta, per_core_inputs, x, W1, b1, W2, b2):
    """Full in_maps for run_bass_kernel_spmd (adds shared tensors).

    x rows are pre-scaled by dinv[src] so gathered rows carry the source
    normalization; weights/biases fold in the 0.5 layer average.
    """
    import ml_dtypes
    bf = ml_dtypes.bfloat16
    dinv = meta["dinv"]
    npc = meta["npc"]
    xs = (np.asarray(x, np.float32) * dinv[:, None]).astype(bf)
    xs = np.ascontiguousarray(xs)
    shared = {
        "x": xs,
        "W1": np.ascontiguousarray((0.5 * np.asarray(W1, np.float32)).astype(bf)),
        "W2": np.ascontiguousarray((0.5 * np.asarray(W2, np.float32)).astype(bf)),
        "b1": (0.5 * np.asarray(b1, np.float32)).astype(bf).reshape(1, D),
        "b2": (0.5 * np.asarray(b2, np.float32)).astype(bf).reshape(1, D),
        "iota": np.ascontiguousarray(np.broadcast_to(
            np.tile(np.arange(128, dtype=np.float32), GF),
            (128, GF * 128)).astype(bf)),
        "ident": np.ascontiguousarray(np.eye(128, dtype=np.float32).astype(bf)),
    }
    maps = []
    for c, ci in enumerate(per_core_inputs):
        m = dict(shared)
        m["x_own"] = np.ascontiguousarray(xs[c * npc:(c + 1) * npc, :])
        m["dstloc"] = np.ascontiguousarray(ci["dstloc"].astype(bf))
        m["dinv_own"] = np.ascontiguousarray(ci["dinv_own"].astype(np.float32))
        m["rdinv"] = np.ascontiguousarray(ci["rdinv"].astype(bf))
        for k, v in ci.items():
            if k.startswith("idx_"):
                m[k] = v
        maps.append(m)
    return maps


# ------------------------------------------------------------------- kernel

def kernel(x, edge_index, W1, b1, W2, b2, _trace=False):
    from concourse.bass_utils import run_bass_kernel_spmd

    x = np.asarray(x)
    n_nodes = x.shape[0]
    meta, pci = host_prep(edge_index, n_nodes, N_CORES)
    nc = build_program(meta)
    in_maps = make_core_inputs(meta, pci, x, W1, b1, W2, b2)
    res = run_bass_kernel_spmd(nc, in_maps, list(range(N_CORES)),
                               trace=_trace)
    out = np.concatenate([res.results[c]["out"] for c in range(N_CORES)],
                         axis=0)
    if _trace:
        return out, res
    return out
